# revision 1
# baseline (speedup 1.0000x reference)
"""AttentionBlock (B=4, C=256, H=W=64) on 8 Trainium2 NeuronCores.

Sharding: data-parallel over (batch, query-half): core i handles batch i//2,
query pixels [half*2048, (half+1)*2048), half = i%2. GroupNorm stats + k/vT
are computed per batch element (duplicated across the pair, cheap); the
O(N^2) attention work is fully sharded 8 ways. No collectives.

Device algorithm per core (pixels m,n in [0,4096), channels c in [0,256)):
  1. GroupNorm stats via bn_stats/bn_aggr per channel + tiny matmuls with 0/1
     group matrices to sum/broadcast across partitions;
     rstd = 1/sqrt(var+eps) via ACT Sqrt + DVE reciprocal.
  2. The normalize step is folded into the convolution weights on device:
     qkv(gn(x)) = (W .* scale_c) x + (W bias_c + b). So the qkv matmuls run
     directly on a f32r copy of x (made during the DMA head, off the stats
     critical path).
  3. k = Wk' x, q = Wq' xh, vT[m,c] = x_m^T Wv'^T (v produced pre-transposed
     so the attention O-matmul needs no transposes).
  4. Attention per 512-wide query chunk, looping 32 key blocks mb:
     S^T[mb,n] (PSUM) <- k_mb^T q;  E = exp(S/16) (ACT PSUM->SBUF, f32r);
     O[c,n] += vT_mb^T E (PSUM accum);  R[n] += ones^T E (M=1 PSUM accum).
     proj runs directly on O (proj is linear, so /R commutes past it);
     Rinv = DVE reciprocal, partition-broadcast by a K=1 ones matmul;
     out = proj(O)*Rinv + xh' (xh' = x_half + all foldable biases, host-made).
All big matmuls run in float32r (TF32-like, ~1.6e-4 rel err, full PE rate).
"""

import numpy as np

B, C, HW = 4, 256, 4096
NH = 2048            # query pixels per core
G, CPG = 32, 8       # groups, channels per group
EPS = 1e-5
MB = HW // 128       # 32 key blocks

_cache = {}


def build_nc():
    """Build (and cache) the Bass module."""
    if "nc" in _cache:
        return _cache["nc"]
    import concourse.tile as tile
    from concourse import bacc, mybir

    f32 = mybir.dt.float32
    f32r = mybir.dt.float32r
    AF = mybir.ActivationFunctionType
    OP = mybir.AluOpType

    nc = bacc.Bacc("TRN2", target_bir_lowering=False, debug=False,
                   enable_asserts=False, num_devices=8)

    # ---- DRAM I/O (host preps everything into device layout) ----
    d_xf = nc.dram_tensor("xf", [128, 2, HW], f32r, kind="ExternalInput")
    d_xh = nc.dram_tensor("xh", [128, 2, NH], f32, kind="ExternalInput")
    d_wq = nc.dram_tensor("wq", [128, 2, C], f32r, kind="ExternalInput")
    d_wk = nc.dram_tensor("wk", [128, 2, C], f32r, kind="ExternalInput")
    d_wv = nc.dram_tensor("wv", [128, 2, C], f32r, kind="ExternalInput")
    d_wp = nc.dram_tensor("wp", [128, 2, C], f32r, kind="ExternalInput")
    d_sb = nc.dram_tensor("sb", [128, 2, 5], f32, kind="ExternalInput")
    d_ag = nc.dram_tensor("ag", [128, 2, G], f32, kind="ExternalInput")
    d_bg = nc.dram_tensor("bg", [G, 2, 128], f32, kind="ExternalInput")
    d_out = nc.dram_tensor("out", [128, 2, NH], f32, kind="ExternalOutput")

    with tile.TileContext(nc) as tc:
        with (
            tc.tile_pool(name="big", bufs=1) as big,
            tc.tile_pool(name="cst", bufs=1) as cst,
            tc.tile_pool(name="wrk", bufs=2) as wrk,
            tc.tile_pool(name="epool", bufs=5) as epool,
            tc.tile_pool(name="gnp", bufs=1) as gnp,
            tc.tile_pool(name="ps_s", bufs=4, space="PSUM") as ps_s,
            tc.tile_pool(name="ps_o", bufs=1, space="PSUM") as ps_o,
            tc.tile_pool(name="ps_t", bufs=1, space="PSUM") as ps_t,
        ):
            # ---- weight/constant loads first (scalar queue; x loads below
            # saturate the sync queue) ----
            # pre-warm ACT table sets before any other ACT work: exp first,
            # sqrt second, so the resident set is sqrt when GN needs it; the
            # attention exp then reloads once, hidden behind qkv matmuls.
            warm = cst.tile([1, 2], f32, tag="warm")
            nc.vector.memset(warm, 1.0)
            nc.scalar.activation(out=warm[:, 0:1], in_=warm[:, 0:1],
                                 func=AF.Exp)
            nc.scalar.activation(out=warm[:, 1:2], in_=warm[:, 1:2],
                                 func=AF.Sqrt)
            smalls = cst.tile([128, 2, 5], f32, tag="smalls")
            nc.scalar.dma_start(out=smalls, in_=d_sb.ap())
            qb = smalls[:, :, 0:1]
            kb = smalls[:, :, 1:2]
            gb = smalls[:, :, 3:4]
            rbias = smalls[:, :, 4:5]
            ag = cst.tile([128, 2, G], f32, tag="ag")
            nc.scalar.dma_start(out=ag, in_=d_ag.ap())
            bg = cst.tile([G, 2, 128], f32, tag="bg")
            nc.scalar.dma_start(out=bg, in_=d_bg.ap())

            # ---- input loads (f32r straight from DRAM; HW rounds on read)
            xfr = big.tile([128, 2, HW], f32r, tag="xfr")
            for ci in range(2):
                for j in range(8):
                    sl = slice(j * 512, (j + 1) * 512)
                    eng = nc.sync if (j % 2 == 0) else nc.scalar
                    eng.dma_start(out=xfr[:, ci, sl], in_=d_xf.ap()[:, ci, sl])
            wall = cst.tile([128, 2, 4 * C], f32r, tag="wall")
            for i, d in enumerate((d_wq, d_wk, d_wv, d_wp)):
                nc.scalar.dma_start(out=wall[:, :, i * C:(i + 1) * C], in_=d.ap())
            xh = big.tile([128, 2, NH], f32, tag="xh")
            xhr = big.tile([128, 2, NH], f32r, tag="xhr")
            xo = big.tile([128, 2, NH], f32, tag="xo")  # x_half + rbias
            for ci in range(2):
                for j in range(2):
                    sl = slice(j * 1024, (j + 1) * 1024)
                    nc.sync.dma_start(out=xh[:, ci, sl], in_=d_xh.ap()[:, ci, sl])
                    nc.vector.tensor_copy(out=xhr[:, ci, sl], in_=xh[:, ci, sl])
                    nc.vector.tensor_scalar(
                        out=xo[:, ci, sl], in0=xh[:, ci, sl],
                        scalar1=rbias[:, ci, :], scalar2=None, op0=OP.add)

            onesc = cst.tile([128, 2], f32, tag="onesc")
            nc.vector.memset(onesc, 1.0)
            epst = cst.tile([G, 1], f32, tag="epst")
            nc.vector.memset(epst, EPS)
            ones_col = cst.tile([128, 1], f32r, tag="ones_col")  # R lhsT
            nc.vector.tensor_copy(out=ones_col, in_=onesc[:, 0:1])
            onesr = cst.tile([1, 128], f32, tag="onesr")
            nc.vector.memset(onesr, 1.0)
            ones_row = cst.tile([1, 128], f32r, tag="ones_row")  # bcast lhsT
            nc.vector.tensor_copy(out=ones_row, in_=onesr)


            # ---- GroupNorm stats ----
            bstat = gnp.tile([128, 2, 8, 6], f32, tag="bstat")
            for ci in range(2):
                for j in range(8):
                    nc.vector.bn_stats(
                        out=bstat[:, ci, j, :],
                        in_=xfr[:, ci, j * 512:(j + 1) * 512])
            stats2 = gnp.tile([128, 2, 2], f32, tag="stats2")  # (mean, E[x^2])
            tmp1 = gnp.tile([128, 1], f32, tag="tmp1")
            for ci in range(2):
                nc.vector.bn_aggr(out=stats2[:, ci, :], in_=bstat[:, ci, :, :])
                nc.vector.tensor_tensor(
                    out=tmp1, in0=stats2[:, ci, 0:1], in1=stats2[:, ci, 0:1],
                    op=OP.mult)
                nc.vector.tensor_tensor(
                    out=stats2[:, ci, 1:2], in0=stats2[:, ci, 1:2], in1=tmp1,
                    op=OP.add)
            # group sums across partitions: [G, 2] = sum_ci ag[ci]^T stats2[ci]
            pg = ps_t.tile([G, 2], f32, tag="t")
            for ci in range(2):
                nc.tensor.matmul(pg, lhsT=ag[:, ci, :], rhs=stats2[:, ci, :],
                                 start=(ci == 0), stop=(ci == 1))
            # ag carries 1/CPG so pg is directly (mean_g, E[x^2]_g)
            pgs = gnp.tile([G, 2], f32, tag="pgs")
            nc.vector.tensor_copy(out=pgs, in_=pg)
            gst = gnp.tile([G, 4], f32, tag="gst")  # mean^2, var, sd, -
            nc.vector.tensor_tensor(out=gst[:, 0:1], in0=pgs[:, 0:1],
                                    in1=pgs[:, 0:1], op=OP.mult)
            nc.vector.tensor_tensor(out=gst[:, 1:2], in0=pgs[:, 1:2],
                                    in1=gst[:, 0:1], op=OP.subtract)
            gfin = gnp.tile([G, 2], f32, tag="gfin")  # (rstd_g, mean_g*rstd_g)
            nc.scalar.activation(out=gst[:, 2:3], in_=gst[:, 1:2],
                                 func=AF.Sqrt, bias=epst)
            nc.vector.reciprocal(out=gfin[:, 0:1], in_=gst[:, 2:3])
            nc.vector.tensor_tensor(out=gfin[:, 1:2], in0=pgs[:, 0:1],
                                    in1=gfin[:, 0:1], op=OP.mult)
            # bg carries gn_w, so pbc = (scale_c, mean_c*scale_c);
            # bias_c = gn_b - mean_c*scale_c
            scbc = gnp.tile([128, 2, 2], f32, tag="scbc")
            for ci in range(2):
                pbc = ps_t.tile([128, 2], f32, tag="t")
                nc.tensor.matmul(pbc, lhsT=bg[:, ci, :], rhs=gfin,
                                 start=True, stop=True)
                nc.vector.tensor_copy(out=scbc[:, ci, 0:1], in_=pbc[:, 0:1])
                nc.vector.tensor_tensor(out=scbc[:, ci, 1:2], in0=gb[:, ci, :],
                                        in1=pbc[:, 1:2], op=OP.subtract)

            # ---- fold GN into conv weights ----
            # 1) unscaled f32r copy; 2) bias matmuls on unscaled W;
            # 3) scale q/k/v weights in place: W' = W .* scale_c (per c_in)
            wqs = wall[:, :, 0 * C:1 * C]
            wks = wall[:, :, 1 * C:2 * C]
            wvs = wall[:, :, 2 * C:3 * C]
            wp = wall[:, :, 3 * C:4 * C]
            # scale weights in place FIRST (gates the qkv matmuls);
            # the bias chain below then uses scaled W with b/s instead
            for ci in range(2):
                nc.vector.tensor_scalar(
                    out=wall[:, ci, 0:3 * C], in0=wall[:, ci, 0:3 * C],
                    scalar1=scbc[:, ci, 0:1], scalar2=None, op0=OP.mult)
            # bsr = bias_c / scale_c (so W' @ bsr == W @ bias_c); 2 copies:
            # f32r matmuls need an even moving free dim
            sinv = gnp.tile([128, 2, 1], f32, tag="sinv")
            bsr = gnp.tile([128, 2, 1], f32, tag="bsr")
            bcr = cst.tile([128, 2, 2], f32r, tag="bcr")
            for ci in range(2):
                nc.vector.reciprocal(out=sinv[:, ci, :], in_=scbc[:, ci, 0:1])
                nc.vector.tensor_tensor(out=bsr[:, ci, :], in0=scbc[:, ci, 1:2],
                                        in1=sinv[:, ci, :], op=OP.mult)
                nc.vector.tensor_copy(out=bcr[:, ci, 0:1], in_=bsr[:, ci, :])
                nc.vector.tensor_copy(out=bcr[:, ci, 1:2], in_=bsr[:, ci, :])
            # effective channel biases: qb2/kb2 = b + W bias_c (unscaled W)
            bias2 = gnp.tile([128, 2, 2], f32, tag="bias2")  # cols: qb2, kb2
            for wi, wsl in enumerate((wqs, wks)):
                for cb in range(2):
                    pbias = ps_t.tile([128, 2], f32, tag="t")
                    for ci in range(2):
                        nc.tensor.matmul(
                            pbias,
                            lhsT=wsl[:, ci, cb * 128:(cb + 1) * 128],
                            rhs=bcr[:, ci, :], start=(ci == 0), stop=(ci == 1))
                    nc.vector.tensor_tensor(
                        out=bias2[:, cb, wi:wi + 1], in0=pbias[:, 0:1],
                        in1=(qb if wi == 0 else kb)[:, cb, :], op=OP.add)
            # v bias along FREE dim: vb2[1, c_out] = bias_c^T Wv (unscaled)
            pvb = ps_t.tile([1, 512], f32, tag="t")
            for ci in range(2):
                nc.tensor.matmul(pvb[:, 0:C], lhsT=bcr[:, ci, 0:1],
                                 rhs=wvs[:, ci, :], start=(ci == 0),
                                 stop=(ci == 1))
            vb2r = gnp.tile([1, C], f32r, tag="vb2r")
            nc.scalar.copy(out=vb2r, in_=pvb[:, 0:C])
            vb2b = gnp.tile([128, C], f32, tag="vb2b")
            pvbb = ps_t.tile([128, 512], f32, tag="t")
            nc.tensor.matmul(pvbb[:, 0:C], lhsT=ones_row, rhs=vb2r,
                             start=True, stop=True)
            nc.scalar.copy(out=vb2b, in_=pvbb[:, 0:C])

            nc.scalar.activation(out=warm[:, 0:1], in_=warm[:, 0:1],
                                 func=AF.Exp)
            # ---- qkv matmuls (on x directly; weights carry the GN fold) ----
            kt = big.tile([128, 2, HW], f32r, tag="xfr2")
            for cb in range(2):
                for j in range(8):
                    sl = slice(j * 512, (j + 1) * 512)
                    pk = ps_s.tile([128, 512], f32, tag="s")
                    for ci in range(2):
                        nc.tensor.matmul(
                            pk, lhsT=wks[:, ci, cb * 128:(cb + 1) * 128],
                            rhs=xfr[:, ci, sl], start=(ci == 0), stop=(ci == 1))
                    nc.scalar.activation(out=kt[:, cb, sl], in_=pk,
                                         func=AF.Identity,
                                         bias=bias2[:, cb, 1:2])
            vT = big.tile([128, MB, C], f32r, tag="vT")
            for mb in range(MB):
                msl = slice(mb * 128, (mb + 1) * 128)
                pv = ps_s.tile([128, 512], f32, tag="s")
                for ci in range(2):
                    nc.tensor.matmul(pv[:, 0:C], lhsT=xfr[:, ci, msl],
                                     rhs=wvs[:, ci, :],
                                     start=(ci == 0), stop=(ci == 1))
                nc.vector.tensor_tensor(out=vT[:, mb, :], in0=pv[:, 0:C],
                                        in1=vb2b, op=OP.add)
            qt = big.tile([128, 2, NH], f32r, tag="qt")
            for cb in range(2):
                for j in range(4):
                    sl = slice(j * 512, (j + 1) * 512)
                    pq = ps_s.tile([128, 512], f32, tag="s")
                    for ci in range(2):
                        nc.tensor.matmul(
                            pq, lhsT=wqs[:, ci, cb * 128:(cb + 1) * 128],
                            rhs=xhr[:, ci, sl], start=(ci == 0), stop=(ci == 1))
                    nc.scalar.activation(out=qt[:, cb, sl], in_=pq,
                                         func=AF.Identity,
                                         bias=bias2[:, cb, 0:1])

            # ---- attention ----
            for j in range(NH // 512):
                sl = slice(j * 512, (j + 1) * 512)
                po = ps_o.tile([128, 3, 512], f32, tag="o")  # O c0, O c1, R
                for mb in range(MB):
                    ps = ps_s.tile([128, 512], f32, tag="s")
                    for ci in range(2):
                        nc.tensor.matmul(
                            ps, lhsT=kt[:, ci, mb * 128:(mb + 1) * 128],
                            rhs=qt[:, ci, sl], start=(ci == 0), stop=(ci == 1))
                    et = epool.tile([128, 512], f32r, tag="et")
                    nc.scalar.activation(out=et, in_=ps, func=AF.Exp,
                                         scale=1.0 / 16.0)
                    for cb in range(2):
                        nc.tensor.matmul(
                            po[:, cb, :],
                            lhsT=vT[:, mb, cb * 128:(cb + 1) * 128],
                            rhs=et, start=(mb == 0), stop=(mb == MB - 1),
                            skip_group_check=True)
                    nc.tensor.matmul(
                        po[0:1, 2, :], lhsT=ones_col, rhs=et,
                        start=(mb == 0), stop=(mb == MB - 1),
                        skip_group_check=True)
                # Free po fast: copy R and both O banks out immediately
                # (ACT + DVE in parallel); the slow single-lane reciprocal
                # then runs on the SBUF copy without holding po.
                rsb = wrk.tile([1, 512], f32, tag="rsb")
                nc.vector.tensor_copy(out=rsb, in_=po[0:1, 2, :])
                onorm = wrk.tile([128, 2, 512], f32r, tag="onorm")
                nc.vector.tensor_copy(out=onorm[:, 0, :], in_=po[:, 0, :])
                nc.vector.tensor_copy(out=onorm[:, 1, :], in_=po[:, 1, :])
                rinv = wrk.tile([1, 512], f32r, tag="rinv")
                with nc.allow_low_precision(reason="f32r is full fp32 storage"):
                    nc.vector.reciprocal(out=rinv, in_=rsb)
                last = (j == NH // 512 - 1)
                if last:
                    # final chunk: nothing left to overlap with, so keep PE's
                    # last matmuls off the slow reciprocal chain -- proj runs
                    # first (into ps_t + po's freed O bank), broadcast goes to
                    # po's freed R bank.
                    pps = []
                    for cb in range(2):
                        if cb == 0:
                            pp = ps_t.tile([128, 512], f32, tag="t",
                                           name="pp_last")
                        else:
                            pp = po[:, 1, :]
                        for ci in range(2):
                            nc.tensor.matmul(
                                pp, lhsT=wp[:, ci, cb * 128:(cb + 1) * 128],
                                rhs=onorm[:, ci, :], start=(ci == 0),
                                stop=(ci == 1), skip_group_check=True)
                        pps.append(pp)
                    nc.tensor.matmul(po[:, 2, :], lhsT=ones_row, rhs=rinv,
                                     start=True, stop=True,
                                     skip_group_check=True)
                    rb = wrk.tile([128, 512], f32, tag="rb")
                    nc.vector.tensor_copy(out=rb, in_=po[:, 2, :])
                    for cb in range(2):
                        outt = wrk.tile([128, 512], f32, tag="outt")
                        nc.vector.tensor_tensor(out=outt, in0=pps[cb], in1=rb,
                                                op=OP.mult)
                        nc.vector.tensor_tensor(out=outt, in0=outt,
                                                in1=xo[:, cb, sl], op=OP.add)
                        nc.sync.dma_start(out=d_out.ap()[:, cb, sl], in_=outt)
                else:
                    pbx = ps_t.tile([128, 512], f32, tag="t")
                    nc.tensor.matmul(pbx, lhsT=ones_row, rhs=rinv,
                                     start=True, stop=True)
                    rb = wrk.tile([128, 512], f32, tag="rb")
                    nc.scalar.copy(out=rb, in_=pbx)
                    for cb in range(2):
                        pp = ps_t.tile([128, 512], f32, tag="t")
                        for ci in range(2):
                            nc.tensor.matmul(
                                pp, lhsT=wp[:, ci, cb * 128:(cb + 1) * 128],
                                rhs=onorm[:, ci, :], start=(ci == 0),
                                stop=(ci == 1))
                        outt = wrk.tile([128, 512], f32, tag="outt")
                        nc.vector.tensor_tensor(out=outt, in0=pp, in1=rb,
                                                op=OP.mult)
                        nc.vector.tensor_tensor(out=outt, in0=outt,
                                                in1=xo[:, cb, sl], op=OP.add)
                        nc.sync.dma_start(out=d_out.ap()[:, cb, sl], in_=outt)

    nc.compile()
    _cache["nc"] = nc
    return nc


def _prep_maps(x, gn_w, gn_b, qkv_w, qkv_b, proj_w, proj_b):
    """Host-side sharding + layout prep. Returns list of 8 in_maps."""
    x = np.asarray(x, np.float32)
    qkv_w = np.asarray(qkv_w, np.float32)
    qkv_b = np.asarray(qkv_b, np.float32)
    proj_w = np.asarray(proj_w, np.float32)
    proj_b = np.asarray(proj_b, np.float32)
    gn_w = np.asarray(gn_w, np.float32)
    gn_b = np.asarray(gn_b, np.float32)

    def chunked(a):  # [256, ...] -> [128, 2, ...]
        return np.ascontiguousarray(a.reshape(2, 128, *a.shape[1:]).transpose(
            1, 0, *range(2, a.ndim + 1)))

    wq = chunked(qkv_w[0:C].T.copy())          # [c_in, c_out] -> [128,2,C]
    wk = chunked(qkv_w[C:2 * C].T.copy())
    wv = chunked(qkv_w[2 * C:3 * C].T.copy())
    wp = chunked(proj_w.T.copy())
    rbias = proj_w @ qkv_b[2 * C:3 * C] + proj_b   # v-bias fold + proj bias
    smalls = np.stack([qkv_b[0:C], qkv_b[C:2 * C], gn_w, gn_b, rbias], axis=1)
    smalls = chunked(smalls)

    cidx = np.arange(C)
    ag_full = (cidx[:, None] // CPG == np.arange(G)[None, :]).astype(np.float32)
    ag = chunked(ag_full / CPG)                     # [128, 2, G], carries 1/8
    bg_full = ag_full * gn_w[:, None]               # carries gn_w
    bg = np.ascontiguousarray(
        bg_full.reshape(2, 128, G).transpose(2, 0, 1))  # [G, 2, 128]

    maps = []
    for core in range(8):
        b, half = core // 2, core % 2
        xf = x[b].reshape(C, HW)
        xh = xf[:, half * NH:(half + 1) * NH]
        maps.append({
            "xf": chunked(xf), "xh": chunked(xh),
            "wq": wq, "wk": wk, "wv": wv, "wp": wp,
            "sb": smalls, "ag": ag, "bg": bg,
        })
    return maps


def kernel(x, gn_w, gn_b, qkv_w, qkv_b, proj_w, proj_b):
    import concourse.bass_utils as bu
    nc = build_nc()
    maps = _prep_maps(x, gn_w, gn_b, qkv_w, qkv_b, proj_w, proj_b)
    res = bu.run_bass_kernel_spmd(nc, maps, core_ids=list(range(8)))
    out = np.empty((B, C, HW), np.float32)
    for core in range(8):
        b, half = core // 2, core % 2
        o = res.results[core]["out"]                # [128, 2, NH]
        out[b, :, half * NH:(half + 1) * NH] = \
            o.transpose(1, 0, 2).reshape(C, NH)
    return out.reshape(B, C, 64, 64)



# revision 9
# speedup vs baseline: 1.5141x; 1.5141x over previous
"""AttentionBlock (B=4, C=256, H=W=64) on 8 Trainium2 NeuronCores.

Sharding: data-parallel over (batch, query-half): core i handles batch i//2,
query pixels [half*2048, (half+1)*2048), half = i%2. GroupNorm stats + k/vT
are computed per batch element (duplicated across the pair, cheap); the
O(N^2) attention work is fully sharded 8 ways. No collectives.

v2: all large matmuls run in fp8e4 (e4m3) with MatmulPerfMode.DoubleRow:
K=256 contracts in one instruction at 0.5 cycles/row (157 TF/s), ~3x the
f32r rate of v1. Weights are pre-scaled by 16 on the fp8 cast so W*s values
sit mid-range in e4m3; the 1/16 is folded into the PSUM-drain ops. exp runs
on ACT in [128, 1024] chunks (one per key-block pair) straight into fp8
attention weights; softmax normalization (1/R) is applied after the O
matmuls (bf16 reciprocal of a PE-broadcast R row). The residual + proj 1/16
correction fuse into one scalar_tensor_tensor per output tile. Host ships x
in fp8 (keys rolled so each core's query half comes first), the residual
base x+rbias in f32, and weights in bf16 (GN-scale folded + cast to fp8 on
device after bn_stats).
"""

import numpy as np

B, C, HW = 4, 256, 4096
NH = 2048            # query pixels per core
G, CPG = 32, 8       # groups, channels per group
EPS = 1e-5
MB = HW // 128       # 32 key blocks
NP = MB // 2         # 16 key-block pairs
SW = 16.0            # fp8 weight pre-scale

_cache = {}


def build_nc():
    """Build (and cache) the Bass module."""
    if "nc" in _cache:
        return _cache["nc"]
    import concourse.tile as tile
    from concourse import bacc, mybir

    f32 = mybir.dt.float32
    bf16 = mybir.dt.bfloat16
    f8 = mybir.dt.float8e4
    AF = mybir.ActivationFunctionType
    OP = mybir.AluOpType
    PM = mybir.MatmulPerfMode

    nc = bacc.Bacc("TRN2", target_bir_lowering=False, debug=False,
                   enable_asserts=False, num_devices=8)

    # ---- DRAM I/O (host preps everything into device layout) ----
    d_xf = nc.dram_tensor("xf", [128, 2, HW], f8, kind="ExternalInput")
    d_xo = nc.dram_tensor("xo", [128, 2, NH], f32, kind="ExternalInput")
    d_w = nc.dram_tensor("w", [128, 2, 4 * C], bf16, kind="ExternalInput")
    d_sb = nc.dram_tensor("sb", [128, 2, 3], f32, kind="ExternalInput")
    d_ag = nc.dram_tensor("ag", [128, 2, G], f32, kind="ExternalInput")
    d_bg = nc.dram_tensor("bg", [G, 2, 128], f32, kind="ExternalInput")
    d_out = nc.dram_tensor("out", [128, 2, NH], f32, kind="ExternalOutput")

    with tile.TileContext(nc) as tc:
        with (
            tc.tile_pool(name="big", bufs=1) as big,
            tc.tile_pool(name="cst", bufs=1) as cst,
            tc.tile_pool(name="wrk", bufs=2) as wrk,
            tc.tile_pool(name="epool", bufs=3) as epool,
            tc.tile_pool(name="gnp", bufs=1) as gnp,
            tc.tile_pool(name="ps_s", bufs=2, space="PSUM") as ps_s,
            tc.tile_pool(name="ps_o", bufs=1, space="PSUM") as ps_o,
            tc.tile_pool(name="ps_r", bufs=1, space="PSUM") as ps_r,
            tc.tile_pool(name="ps_x", bufs=1, space="PSUM") as ps_x,
        ):
            # pre-warm ACT tables: exp first, sqrt second (sqrt resident for
            # GN; exp reloads after GN, before the attention stream).
            warm = cst.tile([1, 2], f32, tag="warm")
            nc.vector.memset(warm, 1.0)
            nc.scalar.activation(out=warm[:, 0:1], in_=warm[:, 0:1],
                                 func=AF.Exp)
            nc.scalar.activation(out=warm[:, 1:2], in_=warm[:, 1:2],
                                 func=AF.Sqrt)
            smalls = cst.tile([128, 2, 3], f32, tag="smalls")
            nc.scalar.dma_start(out=smalls, in_=d_sb.ap())
            qb = smalls[:, :, 0:1]
            kb = smalls[:, :, 1:2]
            gb = smalls[:, :, 2:3]
            ag = cst.tile([128, 2, G], f32, tag="ag")
            nc.scalar.dma_start(out=ag, in_=d_ag.ap())
            bg = cst.tile([G, 2, 128], f32, tag="bg")
            nc.scalar.dma_start(out=bg, in_=d_bg.ap())
            wall = cst.tile([128, 2, 4 * C], bf16, tag="wall")
            nc.scalar.dma_start(out=wall, in_=d_w.ap())

            # x in fp8, 8 chunk DMAs alternating queues
            xf8 = big.tile([128, 2, HW], f8, tag="xf8")
            for ci in range(2):
                for j in range(4):
                    sl = slice(j * 1024, (j + 1) * 1024)
                    eng = nc.sync if (j % 2 == 0) else nc.scalar
                    eng.dma_start(out=xf8[:, ci, sl], in_=d_xf.ap()[:, ci, sl])
            xo = big.tile([128, 2, NH], f32, tag="xo")
            for ci in range(2):
                nc.sync.dma_start(out=xo[:, ci, :], in_=d_xo.ap()[:, ci, :])

            onesc = cst.tile([128, 2], f32, tag="onesc")
            nc.vector.memset(onesc, 1.0)
            negc = cst.tile([128, 1], f32, tag="negc")  # softmax logit shift
            nc.vector.memset(negc, -3.0)
            epst = cst.tile([G, 1], f32, tag="epst")
            nc.vector.memset(epst, EPS)
            # R lhsT (DoubleRow, M=128: R lands pre-broadcast on all rows)
            ones8 = cst.tile([128, 2, 128], f8, tag="ones8")
            nc.vector.memset(ones8, 1.0)
            onesr = cst.tile([1, 128], bf16, tag="onesr")  # bcast lhsT
            nc.vector.memset(onesr, 1.0)

            # ---- GroupNorm stats (on fp8 x) ----
            bstat = gnp.tile([128, 2, 8, 6], f32, tag="bstat")
            for ci in range(2):
                for j in range(8):
                    nc.vector.bn_stats(
                        out=bstat[:, ci, j, :],
                        in_=xf8[:, ci, j * 512:(j + 1) * 512])
            stats2 = gnp.tile([128, 2, 2], f32, tag="stats2")  # (mean, E[x^2])
            tmp1 = gnp.tile([128, 1], f32, tag="tmp1")
            for ci in range(2):
                nc.vector.bn_aggr(out=stats2[:, ci, :], in_=bstat[:, ci, :, :])
                nc.vector.tensor_tensor(
                    out=tmp1, in0=stats2[:, ci, 0:1], in1=stats2[:, ci, 0:1],
                    op=OP.mult)
                nc.vector.tensor_tensor(
                    out=stats2[:, ci, 1:2], in0=stats2[:, ci, 1:2], in1=tmp1,
                    op=OP.add)
            # group sums across partitions: [G, 2] = sum_ci ag[ci]^T stats2[ci]
            pg = ps_x.tile([G, 2], f32, tag="x")
            for ci in range(2):
                nc.tensor.matmul(pg, lhsT=ag[:, ci, :], rhs=stats2[:, ci, :],
                                 start=(ci == 0), stop=(ci == 1))
            # ag carries 1/CPG so pg is directly (mean_g, E[x^2]_g)
            pgs = gnp.tile([G, 2], f32, tag="pgs")
            nc.vector.tensor_copy(out=pgs, in_=pg)
            gst = gnp.tile([G, 4], f32, tag="gst")  # mean^2, var, sd, -
            nc.vector.tensor_tensor(out=gst[:, 0:1], in0=pgs[:, 0:1],
                                    in1=pgs[:, 0:1], op=OP.mult)
            nc.vector.tensor_tensor(out=gst[:, 1:2], in0=pgs[:, 1:2],
                                    in1=gst[:, 0:1], op=OP.subtract)
            gfin = gnp.tile([G, 2], f32, tag="gfin")  # (rstd_g, mean_g*rstd_g)
            nc.scalar.activation(out=gst[:, 2:3], in_=gst[:, 1:2],
                                 func=AF.Sqrt, bias=epst)
            nc.vector.reciprocal(out=gfin[:, 0:1], in_=gst[:, 2:3])
            nc.vector.tensor_tensor(out=gfin[:, 1:2], in0=pgs[:, 0:1],
                                    in1=gfin[:, 0:1], op=OP.mult)
            # bg carries gn_w, so pbc = (scale_c, mean_c*scale_c);
            # bias_c = gn_b - mean_c*scale_c
            scbc = gnp.tile([128, 2, 2], f32, tag="scbc")
            for ci in range(2):
                pbc = ps_x.tile([128, 2], f32, tag="x")
                nc.tensor.matmul(pbc, lhsT=bg[:, ci, :], rhs=gfin,
                                 start=True, stop=True)
                nc.vector.tensor_copy(out=scbc[:, ci, 0:1], in_=pbc[:, 0:1])
                nc.vector.tensor_tensor(out=scbc[:, ci, 1:2], in0=gb[:, ci, :],
                                        in1=pbc[:, 1:2], op=OP.subtract)

            # ---- channel biases from the GN fold (on unscaled bf16 W) ----
            bcr = gnp.tile([128, 2, 2], bf16, tag="bcr")  # bias_c, 2 copies
            for ci in range(2):
                nc.vector.tensor_copy(out=bcr[:, ci, 0:1],
                                      in_=scbc[:, ci, 1:2])
                nc.vector.tensor_copy(out=bcr[:, ci, 1:2],
                                      in_=scbc[:, ci, 1:2])
            # bias2[:, cb, wi] = qb/kb + W_wi @ bias_c
            bias2 = gnp.tile([128, 2, 2], f32, tag="bias2")
            for wi in range(2):
                for cb in range(2):
                    pbias = ps_x.tile([128, 2], f32, tag="x")
                    for ci in range(2):
                        nc.tensor.matmul(
                            pbias,
                            lhsT=wall[:, ci, wi * C + cb * 128:
                                      wi * C + (cb + 1) * 128],
                            rhs=bcr[:, ci, :], start=(ci == 0), stop=(ci == 1))
                    nc.vector.tensor_tensor(
                        out=bias2[:, cb, wi:wi + 1], in0=pbias[:, 0:1],
                        in1=(qb if wi == 0 else kb)[:, cb, :], op=OP.add)
            # v bias along FREE dim: vb2[1, c_out] = bias_c^T Wv
            pvb = ps_x.tile([1, 512], f32, tag="x")
            for ci in range(2):
                nc.tensor.matmul(pvb[:, 0:C], lhsT=bcr[:, ci, 0:1],
                                 rhs=wall[:, ci, 2 * C:3 * C],
                                 start=(ci == 0), stop=(ci == 1))
            vb2r = gnp.tile([1, C], bf16, tag="vb2r")
            nc.scalar.copy(out=vb2r, in_=pvb[:, 0:C])
            pvbb = ps_x.tile([128, 512], f32, tag="x")
            nc.tensor.matmul(pvbb[:, 0:C], lhsT=onesr, rhs=vb2r,
                             start=True, stop=True)
            vb2b = gnp.tile([128, 2, C], f32, tag="vb2b")
            nc.vector.tensor_copy(out=vb2b[:, 0, :], in_=pvbb[:, 0:C])
            nc.vector.tensor_copy(out=vb2b[:, 1, :], in_=pvbb[:, 0:C])

            # ---- fp8 weights: W8 = W * scale_c * 16 (q,k,v), W * 16 (proj)
            w8 = cst.tile([128, 2, 4 * C], f8, tag="w8")
            for ci in range(2):
                nc.vector.tensor_scalar(
                    out=w8[:, ci, 0:3 * C], in0=wall[:, ci, 0:3 * C],
                    scalar1=scbc[:, ci, 0:1], scalar2=SW,
                    op0=OP.mult, op1=OP.mult)
                nc.vector.tensor_scalar(
                    out=w8[:, ci, 3 * C:4 * C], in0=wall[:, ci, 3 * C:4 * C],
                    scalar1=SW, scalar2=None, op0=OP.mult)

            # reload exp before the attention stream
            nc.scalar.activation(out=warm[:, 0:1], in_=warm[:, 0:1],
                                 func=AF.Exp)

            kt = big.tile([128, 2, HW], f8, tag="kt")
            qt = big.tile([128, 2, NH], f8, tag="qt")
            vT = big.tile([128, MB, C], f8, tag="vT")

            def qt_unit(j):  # 512 queries
                sl = slice(j * 512, (j + 1) * 512)
                pq = ps_s.tile([128, 2, 512], f32, tag="s", name=f"pq{j}")
                for cb in range(2):
                    nc.tensor.matmul(
                        pq[:, cb, :],
                        lhsT=w8[:, :, cb * 128:(cb + 1) * 128],
                        rhs=xf8[:, :, sl], start=True, stop=True,
                        perf_mode=PM.DoubleRow)
                for cb in range(2):
                    nc.vector.tensor_scalar(
                        out=qt[:, cb, sl], in0=pq[:, cb, :],
                        scalar1=1.0 / SW, scalar2=bias2[:, cb, 0:1],
                        op0=OP.mult, op1=OP.add)

            def kt_unit(u):  # 512 pixels
                sl = slice(u * 512, (u + 1) * 512)
                pk = ps_s.tile([128, 2, 512], f32, tag="s", name=f"pk{u}")
                for cb in range(2):
                    nc.tensor.matmul(
                        pk[:, cb, :],
                        lhsT=w8[:, :, C + cb * 128:C + (cb + 1) * 128],
                        rhs=xf8[:, :, sl], start=True, stop=True,
                        perf_mode=PM.DoubleRow)
                for cb in range(2):
                    nc.vector.tensor_scalar(
                        out=kt[:, cb, sl], in0=pk[:, cb, :],
                        scalar1=1.0 / SW, scalar2=bias2[:, cb, 1:2],
                        op0=OP.mult, op1=OP.add)

            def vt_unit(u):  # 4 key blocks = 2 pairs
                pv = ps_s.tile([128, 2, 512], f32, tag="s", name=f"pv{u}")
                for h in range(2):
                    for par in range(2):
                        mb = 4 * u + 2 * h + par
                        msl = slice(mb * 128, (mb + 1) * 128)
                        nc.tensor.matmul(
                            pv[:, par, h * 256:h * 256 + 256],
                            lhsT=xf8[:, :, msl],
                            rhs=w8[:, :, 2 * C:3 * C], start=True, stop=True,
                            perf_mode=PM.DoubleRow)
                for h in range(2):
                    p = 2 * u + h
                    nc.vector.scalar_tensor_tensor(
                        out=vT[:, 2 * p:2 * p + 2, :],
                        in0=pv[:, :, h * 256:h * 256 + 256],
                        scalar=1.0 / SW, in1=vb2b,
                        op0=OP.mult, op1=OP.add)

            def attn_pairs(j, plo, phi):
                """Attention for query chunk j, key pairs [plo, phi)."""
                sl = slice(j * 512, (j + 1) * 512)
                for p in range(plo, phi):
                    sp = ps_s.tile([128, 2, 512], f32, tag="s",
                                   name=f"sp{j}_{p}")
                    for par in range(2):
                        mb = 2 * p + par
                        nc.tensor.matmul(
                            sp[:, par, :],
                            lhsT=kt[:, :, mb * 128:(mb + 1) * 128],
                            rhs=qt[:, :, sl], start=True, stop=True,
                            perf_mode=PM.DoubleRow)
                    # logit shift keeps exp in e4m3 range (max logit ~8:
                    # e^(8-3)=148 < 240); softmax is shift-invariant
                    et = epool.tile([128, 2, 512], f8, tag="et")
                    nc.scalar.activation(out=et, in_=sp, func=AF.Exp,
                                         scale=1.0 / SW, bias=negc)
                    for cb in range(2):
                        nc.tensor.matmul(
                            po[:, cb, :],
                            lhsT=vT[:, 2 * p:2 * p + 2,
                                    cb * 128:(cb + 1) * 128],
                            rhs=et, start=(p == 0), stop=(p == NP - 1),
                            perf_mode=PM.DoubleRow, skip_group_check=True)
                    nc.tensor.matmul(
                        pr, lhsT=ones8, rhs=et,
                        start=(p == 0), stop=(p == NP - 1),
                        perf_mode=PM.DoubleRow, skip_group_check=True)

            def chunk_tail(j):
                """Normalize, project, add residual, store chunk j."""
                sl = slice(j * 512, (j + 1) * 512)
                onorm = wrk.tile([128, 2, 512], bf16, tag="onorm")
                nc.vector.tensor_copy(out=onorm, in_=po)  # frees po
                rb = wrk.tile([128, 512], bf16, tag="rb")
                with nc.allow_low_precision(reason="bf16 softmax denom"):
                    nc.vector.reciprocal(out=rb, in_=pr)  # frees pr
                onormed = wrk.tile([128, 2, 512], f8, tag="onormed")
                for cb in range(2):
                    nc.vector.tensor_tensor(
                        out=onormed[:, cb, :], in0=onorm[:, cb, :], in1=rb,
                        op=OP.mult)
                for co in range(2):
                    pp = ps_x.tile([128, 512], f32, tag="x", name=f"pp{j}_{co}")
                    nc.tensor.matmul(
                        pp, lhsT=w8[:, :, 3 * C + co * 128:
                                    3 * C + (co + 1) * 128],
                        rhs=onormed, start=True, stop=True,
                        perf_mode=PM.DoubleRow)
                    outt = wrk.tile([128, 512], f32, tag="outt")
                    nc.vector.scalar_tensor_tensor(
                        out=outt, in0=pp, scalar=1.0 / SW,
                        in1=xo[:, co, sl], op0=OP.mult, op1=OP.add)
                    nc.sync.dma_start(out=d_out.ap()[:, co, sl], in_=outt)

            # ---- chunk 0: interleave k/v production with attention ----
            qt_unit(0)
            po = ps_o.tile([128, 2, 512], f32, tag="o", name="po0")
            pr = ps_r.tile([128, 512], f32, tag="r", name="pr0")
            for u in range(8):
                kt_unit(u)
                vt_unit(u)
                attn_pairs(0, 2 * u, 2 * u + 2)
            for j in range(1, 4):
                qt_unit(j)
            chunk_tail(0)

            # ---- chunks 1..3 ----
            for j in range(1, 4):
                po = ps_o.tile([128, 2, 512], f32, tag="o", name=f"po{j}")
                pr = ps_r.tile([128, 512], f32, tag="r", name=f"pr{j}")
                attn_pairs(j, 0, NP)
                chunk_tail(j)

    nc.compile()
    _cache["nc"] = nc
    return nc


def _prep_maps(x, gn_w, gn_b, qkv_w, qkv_b, proj_w, proj_b):
    """Host-side sharding + layout prep. Returns list of 8 in_maps."""
    import ml_dtypes
    f8 = ml_dtypes.float8_e4m3
    bf16 = ml_dtypes.bfloat16

    x = np.asarray(x, np.float32)
    qkv_w = np.asarray(qkv_w, np.float32)
    qkv_b = np.asarray(qkv_b, np.float32)
    proj_w = np.asarray(proj_w, np.float32)
    proj_b = np.asarray(proj_b, np.float32)
    gn_w = np.asarray(gn_w, np.float32)
    gn_b = np.asarray(gn_b, np.float32)

    def chunked(a):  # [256, ...] -> [128, 2, ...]
        return np.ascontiguousarray(a.reshape(2, 128, *a.shape[1:]).transpose(
            1, 0, *range(2, a.ndim + 1)))

    wq = qkv_w[0:C].T
    wk = qkv_w[C:2 * C].T
    wv = qkv_w[2 * C:3 * C].T
    wp = proj_w.T
    wall = chunked(np.concatenate([wq, wk, wv, wp], axis=1)).astype(bf16)
    rbias = proj_w @ qkv_b[2 * C:3 * C] + proj_b   # v-bias fold + proj bias
    smalls = chunked(np.stack([qkv_b[0:C], qkv_b[C:2 * C], gn_b], axis=1))

    cidx = np.arange(C)
    ag_full = (cidx[:, None] // CPG == np.arange(G)[None, :]).astype(np.float32)
    ag = chunked(ag_full / CPG)                     # [128, 2, G], carries 1/8
    bg_full = ag_full * gn_w[:, None]               # carries gn_w
    bg = np.ascontiguousarray(
        bg_full.reshape(2, 128, G).transpose(2, 0, 1))  # [G, 2, 128]

    maps = []
    for core in range(8):
        b, half = core // 2, core % 2
        xflat = x[b].reshape(C, HW)
        own = xflat[:, half * NH:(half + 1) * NH]
        other = xflat[:, (1 - half) * NH:(2 - half) * NH]
        xroll = np.concatenate([own, other], axis=1)  # own queries first
        xo = own + rbias[:, None]
        maps.append({
            "xf": chunked(xroll).astype(f8),
            "xo": chunked(xo),
            "w": wall, "sb": smalls, "ag": ag, "bg": bg,
        })
    return maps


def kernel(x, gn_w, gn_b, qkv_w, qkv_b, proj_w, proj_b):
    import concourse.bass_utils as bu
    nc = build_nc()
    maps = _prep_maps(x, gn_w, gn_b, qkv_w, qkv_b, proj_w, proj_b)
    res = bu.run_bass_kernel_spmd(nc, maps, core_ids=list(range(8)))
    out = np.empty((B, C, HW), np.float32)
    for core in range(8):
        b, half = core // 2, core % 2
        o = res.results[core]["out"]                # [128, 2, NH]
        out[b, :, half * NH:(half + 1) * NH] = \
            o.transpose(1, 0, 2).reshape(C, NH)
    return out.reshape(B, C, 64, 64)
